# revision 2
# baseline (speedup 1.0000x reference)
"""Trainium2 Bass kernel for CoSFuserMoELayer (top-1 MoE, E=8 experts).

Strategy (expert-parallel, matching the sharding hint):
  - The router is evaluated on host with the exact same jax ops as the
    reference (bitwise-identical routing decisions / balance loss).
  - Tokens are dispatched host-side to their top-1 expert; core e holds
    expert e's weights and processes only the tokens routed to it
    (sparse compute: ~1/8 of the reference's dense FLOPs).
  - Each core runs gelu(x @ w1 + b1) @ w2 + b2 for its token batch in a
    transpose-free layout (tokens are the moving/free dimension for both
    matmuls; activations stay [features x tokens] on-chip).
  - Matmuls run as float32r (TF32-like) at full PE rate.
  - Host gathers per-expert outputs, scales by the router max-prob and
    reassembles the full [B, S, D] output.
"""

import os
import sys

import numpy as np

for _p in ("/opt/trn_rl_repo", "/opt/pypackages"):
    if _p not in sys.path and os.path.isdir(_p):
        sys.path.append(_p)

B, S, D, H, E = 4, 2048, 1024, 4096, 8
T = B * S
P = 128
DK = D // P          # 8  k-tiles over D
HK = H // P          # 32 h-tiles over H
HALF = 2             # split H into halves so hT fits in SBUF
HKH = HK // HALF     # 16 h-tiles per half
TCH = 384            # token chunk (matmul moving/free dim; >=256 keeps f32r at rate 1)

_nc_cache = {}


def build_ffn(C):
    """Bass program: yT[D,C] = (gelu(xT.T @ w1 + b1) @ w2 + b2).T for one expert."""
    import concourse.bacc as bacc
    import concourse.mybir as mybir
    import concourse.tile as tile

    NT = C // TCH
    f32 = mybir.dt.float32
    f32r = mybir.dt.float32r
    gelu = mybir.ActivationFunctionType.Gelu_apprx_tanh

    nc = bacc.Bacc("TRN2", target_bir_lowering=False, debug=False, num_devices=E)
    xT = nc.declare_dram_parameter("xT", [D, C], f32, isOutput=False)
    w1 = nc.declare_dram_parameter("w1", [D, H], f32, isOutput=False)
    b1 = nc.declare_dram_parameter("b1", [H], f32, isOutput=False)
    w2 = nc.declare_dram_parameter("w2", [H, D], f32, isOutput=False)
    b2 = nc.declare_dram_parameter("b2", [D], f32, isOutput=False)
    yT = nc.declare_dram_parameter("yT", [D, C], f32, isOutput=True)

    with tile.TileContext(nc) as tc:
        with (
            tc.tile_pool(name="xpool", bufs=DK) as xpool,
            tc.tile_pool(name="hpool", bufs=HKH * NT) as hpool,
            tc.tile_pool(name="ypool", bufs=DK * NT) as ypool,
            tc.tile_pool(name="w1pool", bufs=3) as w1pool,
            tc.tile_pool(name="w2pool", bufs=2) as w2pool,
            tc.tile_pool(name="bpool", bufs=2) as bpool,
            tc.tile_pool(name="ps1", bufs=5, space="PSUM") as ps1,
            tc.tile_pool(name="ps2", bufs=3, space="PSUM") as ps2,
        ):
            b1sb = bpool.tile([P, HK], f32, tag="b1")
            nc.sync.dma_start(out=b1sb[:], in_=b1.rearrange("(o p) -> p o", p=P))
            b2sb = bpool.tile([P, DK], f32, tag="b2")
            nc.sync.dma_start(out=b2sb[:], in_=b2.rearrange("(o p) -> p o", p=P))

            xts = []
            for k in range(DK):
                xt = xpool.tile([P, C], f32r, tag="x")
                nc.sync.dma_start(
                    out=xt[:], in_=xT[k * P : (k + 1) * P, :].bitcast(f32r)
                )
                xts.append(xt)

            w1r = w1.rearrange("(ko p) h -> p ko h", p=P)  # [P, DK, H]
            w2r = w2.rearrange("(ko p) d -> p ko d", p=P)  # [P, HK, D]

            yts = {}
            for half in range(HALF):
                hts = {}
                # ---- mm1: hT[hh, t] = gelu(w1.T @ xT + b1) for this half of H
                for m in range(HKH):
                    hh = half * HKH + m
                    w1t = w1pool.tile([P, DK, P], f32r, tag="w1")
                    nc.sync.dma_start(
                        out=w1t[:],
                        in_=w1r[:, :, hh * P : (hh + 1) * P].bitcast(f32r),
                    )
                    pss = [
                        ps1.tile([P, TCH], f32, tag="ps1", name=f"ps1_{half}_{m}_{t}")
                        for t in range(NT)
                    ]
                    for k in range(DK):
                        for t in range(NT):
                            nc.tensor.matmul(
                                pss[t][:],
                                w1t[:, k, :],
                                xts[k][:, t * TCH : (t + 1) * TCH],
                                start=(k == 0),
                                stop=(k == DK - 1),
                            )
                    for t in range(NT):
                        ht = hpool.tile([P, TCH], f32r, tag="h")
                        nc.scalar.activation(
                            ht[:], pss[t][:], gelu, bias=b1sb[:, hh : hh + 1]
                        )
                        hts[(m, t)] = ht
                # ---- mm2 (partial sum over this half of H)
                for m in range(DK):
                    w2t = w2pool.tile([P, HKH, P], f32r, tag="w2")
                    nc.sync.dma_start(
                        out=w2t[:],
                        in_=w2r[
                            :, half * HKH : (half + 1) * HKH, m * P : (m + 1) * P
                        ].bitcast(f32r),
                    )
                    pss = [
                        ps2.tile([P, TCH], f32, tag="ps2", name=f"ps2_{half}_{m}_{t}")
                        for t in range(NT)
                    ]
                    for k in range(HKH):
                        for t in range(NT):
                            nc.tensor.matmul(
                                pss[t][:],
                                w2t[:, k, :],
                                hts[(k, t)][:],
                                start=(k == 0),
                                stop=(k == HKH - 1),
                            )
                    for t in range(NT):
                        if half == 0:
                            yt = ypool.tile([P, TCH], f32, tag="y")
                            nc.vector.tensor_add(
                                yt[:],
                                pss[t][:],
                                b2sb[:, m : m + 1].to_broadcast([P, TCH]),
                            )
                            yts[(m, t)] = yt
                        else:
                            yt = yts[(m, t)]
                            nc.vector.tensor_add(yt[:], yt[:], pss[t][:])
            for m in range(DK):
                for t in range(NT):
                    nc.sync.dma_start(
                        out=yT[m * P : (m + 1) * P, t * TCH : (t + 1) * TCH],
                        in_=yts[(m, t)][:],
                    )
    nc.compile()
    return nc


def get_ffn(C):
    if C not in _nc_cache:
        _nc_cache[C] = build_ffn(C)
    return _nc_cache[C]


def _route(x, router_w, router_b):
    """Router replicated bitwise from the reference (same jax ops)."""
    import jax
    import jax.numpy as jnp

    x_flat = jnp.asarray(x).reshape(-1, D)
    logits = x_flat @ jnp.asarray(router_w) + jnp.asarray(router_b)
    probs = jax.nn.softmax(logits, axis=-1)
    max_prob = jnp.max(probs, axis=-1)
    max_idx = jnp.argmax(probs, axis=-1)
    mask = jax.nn.one_hot(max_idx, E, dtype=probs.dtype)
    mean_prob = probs.mean(axis=0)
    expert_usage = mask.sum(axis=0) / mask.sum()
    balance_loss = E * jnp.sum(mean_prob * expert_usage)
    return (
        np.asarray(max_prob),
        np.asarray(max_idx),
        np.asarray(balance_loss),
    )


def kernel(x, router_w, router_b, w1, b1, w2, b2):
    from concourse.bass_utils import run_bass_kernel_spmd

    x = np.ascontiguousarray(np.asarray(x, dtype=np.float32))
    w1 = np.ascontiguousarray(np.asarray(w1, dtype=np.float32))
    b1 = np.ascontiguousarray(np.asarray(b1, dtype=np.float32))
    w2 = np.ascontiguousarray(np.asarray(w2, dtype=np.float32))
    b2 = np.ascontiguousarray(np.asarray(b2, dtype=np.float32))

    max_prob, max_idx, balance_loss = _route(x, router_w, router_b)

    counts = np.bincount(max_idx, minlength=E)
    C = int(max(TCH, -(-counts.max() // TCH) * TCH))
    order = np.argsort(max_idx, kind="stable")
    starts = np.zeros(E + 1, dtype=np.int64)
    starts[1:] = np.cumsum(counts)

    x_flat = x.reshape(T, D)
    tok_lists = []
    in_maps = []
    for e in range(E):
        toks = order[starts[e] : starts[e + 1]]
        tok_lists.append(toks)
        xTe = np.zeros((D, C), dtype=np.float32)
        if len(toks):
            xTe[:, : len(toks)] = x_flat[toks].T
        in_maps.append(
            {
                "xT": xTe,
                "w1": w1[e],
                "b1": b1[e],
                "w2": w2[e],
                "b2": b2[e],
            }
        )

    nc = get_ffn(C)
    res = run_bass_kernel_spmd(nc, in_maps, list(range(E)))

    out_flat = np.zeros((T, D), dtype=np.float32)
    for e in range(E):
        toks = tok_lists[e]
        if len(toks):
            yTe = res.results[e]["yT"]
            out_flat[toks] = yTe[:, : len(toks)].T
    out_flat *= max_prob[:, None]
    return out_flat.reshape(B, S, D), np.float32(balance_loss)
